# revision 7
# baseline (speedup 1.0000x reference)
"""Causal self-attention (GPT-2 small block shape: B=4, T=2048, C=768, H=12, D=64)
on 8 TRN2 NeuronCores.

Sharding: core i handles batch b = i//2 and head-half = i%2 (6 heads each).
No cross-core collectives; the two half-head partial output projections per
batch are summed on the host during unshard (row-parallel c_proj).

Device kernel (per core, all matmuls bf16, fp32 PSUM accumulation):
  1. qkv^T = w^T x^T via TensorE with contraction over C (x^T supplied
     pre-transposed + bf16 by the host); biases folded in as K=1 matmuls
     with a ones vector.  Q is pre-scaled by 1/sqrt(D) host-side.
  2. Per head: S^T[k,q] = K^T.T @ Q^T blocks (causally skipped), exp on
     ScalarE with a large free dim straight out of PSUM into bf16 SBUF,
     diagonal-block masking on VectorE.
  3. AV with V augmented by a ones column -> row sums land in PSUM
     partition 64 for free; normalization = reciprocal (DVE) +
     K=1 broadcast matmul + one tensor-tensor multiply.
  4. Output projection from the y^T layout (contraction over the head dim),
     b_proj added via K=1 matmul (only on half 0; half 1 gets zeros).
"""

import sys

if "/opt/trn_rl_repo" not in sys.path:
    sys.path.insert(0, "/opt/trn_rl_repo")

import numpy as np
import ml_dtypes

import concourse.bass as bass  # noqa: F401  (engine types pulled via nc)
import concourse.mybir as mybir
from concourse import bacc
from concourse.tile import TileContext
from concourse.bass_utils import run_bass_kernel_spmd

BF16 = ml_dtypes.bfloat16

B, T, C = 4, 2048, 768
H, D = 12, 64
NH = 6  # heads per core
P = 128
TC = T // P  # 16 t-chunks of 128
QC = T // 512  # 4 q-chunks of 512
CCH = C // P  # 6 contraction chunks

DT = mybir.dt.bfloat16
F32 = mybir.dt.float32


def build_nc():
    nc = bacc.Bacc()

    xt_d = nc.declare_dram_parameter("xt", [P, CCH, T], DT, isOutput=False)
    wqk_d = nc.declare_dram_parameter("wqk", [P, CCH, 2 * NH * D], DT, isOutput=False)
    bqk_d = nc.declare_dram_parameter("bqk", [P, 2 * NH * D // P], F32, isOutput=False)
    wv_d = nc.declare_dram_parameter("wv", [P, CCH, NH * D], DT, isOutput=False)
    bv_d = nc.declare_dram_parameter("bv", [1, NH * D], DT, isOutput=False)
    wp_d = nc.declare_dram_parameter("wp", [P, NH * D // P, C], DT, isOutput=False)
    bp_d = nc.declare_dram_parameter("bp", [1, C], DT, isOutput=False)
    mask_d = nc.declare_dram_parameter("mask", [P, P], DT, isOutput=False)
    out_d = nc.declare_dram_parameter("out", [T, C], F32, isOutput=True)

    with TileContext(nc) as tc:
        with (
            tc.tile_pool(name="consts", bufs=1) as consts,
            tc.tile_pool(name="work", bufs=3) as work,
            tc.tile_pool(name="outp", bufs=3) as outp,
            tc.tile_pool(name="ps_s", bufs=2, space="PSUM") as ps_s,
            tc.tile_pool(name="ps_misc", bufs=2, space="PSUM") as ps_misc,
        ):
            # ---- load inputs ----
            xt_sb = consts.tile([P, CCH, T], DT)
            nc.sync.dma_start(xt_sb[:, :, 0 : T // 4], xt_d[:, :, 0 : T // 4])
            wqk_sb = consts.tile([P, CCH, 2 * NH * D], DT)
            nc.sync.dma_start(wqk_sb[:], wqk_d[:])
            bqk_sb = consts.tile([P, 2 * NH * D // P], F32)
            nc.sync.dma_start(bqk_sb[:], bqk_d[:])
            for _q in range(1, 4):
                nc.sync.dma_start(
                    xt_sb[:, :, _q * T // 4 : (_q + 1) * T // 4],
                    xt_d[:, :, _q * T // 4 : (_q + 1) * T // 4],
                )
            wv_sb = consts.tile([P, CCH, NH * D], DT)
            nc.sync.dma_start(wv_sb[:], wv_d[:])
            bv_sb = consts.tile([1, NH * D], DT)
            nc.sync.dma_start(bv_sb[:], bv_d[:])
            wp_sb = consts.tile([P, NH * D // P, C], DT)
            nc.sync.dma_start(wp_sb[:], wp_d[:])
            bp_sb = consts.tile([1, C], DT)
            nc.sync.dma_start(bp_sb[:], bp_d[:])
            mask_sb = consts.tile([P, P], DT)
            nc.sync.dma_start(mask_sb[:], mask_d[:])

            bvb = consts.tile([P, NH * D], DT)
            nc.gpsimd.partition_broadcast(bvb[:], bv_sb[:])
            bpb = consts.tile([P, C], DT)
            nc.gpsimd.partition_broadcast(bpb[:], bp_sb[:])

            # Q^T/K^T as head-pair tiles [128, T]: head 2p in partitions 0:64,
            # head 2p+1 in partitions 64:128 (S-gen uses tile_position row=64)
            qtp = [consts.tile([P, T], DT, name=f"qtp{p}", tag=f"qtp{p}") for p in range(NH // 2)]
            ktp = [consts.tile([P, T], DT, name=f"ktp{p}", tag=f"ktp{p}") for p in range(NH // 2)]
            # V per t-chunk, heads side by side with a ones column: [128, 6, 65]
            vt = [consts.tile([P, NH, D + 1], DT, name=f"vt{t}", tag=f"vt{t}") for t in range(TC)]
            for t in range(TC):
                nc.gpsimd.memset(vt[t][:, :, D : D + 1], 1.0)
            # y^T per head-pair [128, T] bf16
            yt = [consts.tile([P, T], DT, name=f"yt{p}", tag=f"yt{p}") for p in range(NH // 2)]

            # ---- QKV helpers ----
            def qk_tile(fc, tcb):
                    # feat chunk fc: 0..2 -> Q pair fc, 3..5 -> K pair fc-3
                    pq = ps_misc.tile([P, 512], F32, tag="pmisc", name="pq")
                    for cc in range(CCH):
                        nc.tensor.matmul(
                            pq[:],
                            wqk_sb[:, cc, fc * P : (fc + 1) * P],
                            xt_sb[:, cc, tcb * 512 : (tcb + 1) * 512],
                            start=(cc == 0),
                            stop=(cc == CCH - 1),
                        )
                    dst = qtp[fc] if fc < 3 else ktp[fc - 3]
                    nc.vector.tensor_scalar_add(
                        dst[:, tcb * 512 : (tcb + 1) * 512],
                        pq[:],
                        bqk_sb[:, fc : fc + 1],
                    )

            def v_chunks(ts_):
                for t in ts_:
                    pv = ps_misc.tile([P, NH * D], F32, tag="pmisc", name="pv")
                    for cc in range(CCH):
                        nc.tensor.matmul(
                            pv[:],
                            xt_sb[:, cc, t * P : (t + 1) * P],
                            wv_sb[:, cc, :],
                            start=(cc == 0),
                            stop=(cc == CCH - 1),
                        )
                    nc.vector.tensor_add(
                        vt[t][:, :, 0:D],
                        pv[:].rearrange("p (h d) -> p h d", d=D),
                        bvb[:].rearrange("p (h d) -> p h d", d=D),
                    )

            # ---- attention for one head pair ----
            def attention(hp, qc):
                    nj = 4 * (qc + 1)  # causal k-chunks for this q block
                    for hi in (0, 1):
                        b0 = 64 * hi
                        av = ps_misc.tile([65, 512], F32, tag="pmisc", name="av")
                        for g in range(nj // 2):  # groups of 2 k-chunks
                            sps = ps_s.tile([P, 1024], F32, tag="s", name="sps")
                            for sub in range(2):
                                j = 2 * g + sub
                                m = max(0, (j - 4 * qc) * P)  # causal trim offset
                                nc.tensor.matmul(
                                    sps[:, sub * 512 + m : (sub + 1) * 512],
                                    ktp[hp][b0 : b0 + 64, j * P : (j + 1) * P],
                                    qtp[hp][b0 : b0 + 64, qc * 512 + m : (qc + 1) * 512],
                                    start=True,
                                    stop=True,
                                )
                            sexp = work.tile([P, 1024], DT, tag="sexp", name="sexp")
                            nc.scalar.activation(
                                sexp[:], sps[:], mybir.ActivationFunctionType.Exp
                            )
                            for sub in range(2):
                                j = 2 * g + sub
                                m = max(0, (j - 4 * qc) * P)
                                if j - 4 * qc >= 0:  # diagonal block: mask
                                    nc.vector.tensor_mul(
                                        sexp[:, sub * 512 + m : sub * 512 + m + P],
                                        sexp[:, sub * 512 + m : sub * 512 + m + P],
                                        mask_sb[:],
                                    )
                                nc.tensor.matmul(
                                    av[:, m:512],
                                    vt[j][:, 2 * hp + hi, :],
                                    sexp[:, sub * 512 + m : (sub + 1) * 512],
                                    start=(j == 0),
                                    stop=(j == nj - 1),
                                    skip_group_check=True,
                                )
                        # normalization: row sums sit in av partition 64
                        sums = work.tile([1, 512], F32, tag="sums", name="sums")
                        nc.vector.tensor_copy(sums[:], av[64:65, :])
                        inv = work.tile([1, 512], F32, tag="inv", name="inv")
                        nc.vector.reciprocal_approx_fast(inv[:], sums[:])
                        invb = work.tile([64, 512], F32, tag="invb", name="invb")
                        nc.gpsimd.partition_broadcast(invb[:], inv[:])
                        nc.vector.tensor_mul(
                            yt[hp][b0 : b0 + 64, qc * 512 : (qc + 1) * 512],
                            av[0:64, :],
                            invb[:],
                        )

            # ---- output projection for a batch of t chunks ----
            def proj(ts_):
                for t in ts_:
                    pp = ps_misc.tile([P, C], F32, tag="pmisc", name="pp")
                    for c0, c1 in ((0, 512), (512, C)):
                        for cp in range(NH * D // P):
                            nc.tensor.matmul(
                                pp[:, c0:c1],
                                yt[cp][:, t * P : (t + 1) * P],
                                wp_sb[:, cp, c0:c1],
                                start=(cp == 0),
                                stop=(cp == NH * D // P - 1),
                            )
                    stg = outp.tile([P, C], F32)
                    nc.vector.tensor_add(stg[:], pp[:], bpb[:])
                    nc.sync.dma_start(out_d[t * P : (t + 1) * P, :], stg[:])

            # ---- schedule: 4 uniform rounds over q blocks ----
            for qc in range(QC):
                for fc in (0, 3, 1, 4, 2, 5):
                    qk_tile(fc, qc)
                v_chunks(range(4 * qc, 4 * qc + 4))
                for hp in range(3):
                    attention(hp, qc)
                proj(range(4 * qc, 4 * qc + 4))

    nc.finalize()
    return nc

def shard_inputs(x, w_attn, b_attn, w_proj, b_proj):
    """Host-side prep: slice per core, transpose x, cast to bf16."""
    scale = 1.0 / np.sqrt(D)
    tril = np.tril(np.ones((P, P), np.float32))
    # mask[k_local, q_local] = 1 where k <= q
    mask = tril.T.astype(BF16)
    in_maps = []
    for core in range(8):
        b, half = divmod(core, 2)
        h0 = half * NH
        cq = slice(h0 * D, (h0 + NH) * D)
        ck = slice(C + h0 * D, C + (h0 + NH) * D)
        cv = slice(2 * C + h0 * D, 2 * C + (h0 + NH) * D)
        wq = (w_attn[:, cq] * scale).astype(BF16)
        wk = w_attn[:, ck].astype(BF16)
        wqk = np.concatenate([wq, wk], axis=1)  # [C, 768]
        bqk = np.concatenate([(b_attn[cq] * scale), b_attn[ck]], axis=0).astype(
            np.float32
        )
        bqk_col = np.ascontiguousarray(bqk.reshape(2 * NH * D // P, P).T)
        wv = w_attn[:, cv].astype(BF16)
        bv = b_attn[cv].astype(BF16)[None, :]
        wp = w_proj[h0 * D : (h0 + NH) * D, :].astype(BF16)
        bp = (b_proj if half == 0 else np.zeros_like(b_proj)).astype(BF16)[None, :]
        xt = np.ascontiguousarray(x[b].T).astype(BF16)  # [C, T]
        in_maps.append(
            {
                "xt": np.ascontiguousarray(
                    xt.reshape(CCH, P, T).transpose(1, 0, 2)
                ),
                "wqk": np.ascontiguousarray(
                    wqk.reshape(CCH, P, 2 * NH * D).transpose(1, 0, 2)
                ),
                "bqk": bqk_col,
                "wv": np.ascontiguousarray(
                    wv.reshape(CCH, P, NH * D).transpose(1, 0, 2)
                ),
                "bv": bv,
                "wp": np.ascontiguousarray(
                    wp.reshape(NH * D // P, P, C).transpose(1, 0, 2)
                ),
                "bp": bp,
                "mask": mask,
            }
        )
    return in_maps


_NC = None


def _get_nc():
    global _NC
    if _NC is None:
        _NC = build_nc()
    return _NC


def run_sharded(in_maps, trace=False, **kw):
    nc = _get_nc()
    return run_bass_kernel_spmd(nc, in_maps, core_ids=list(range(8)), trace=trace, **kw)


def gather(results):
    out = np.zeros((B, T, C), np.float32)
    for core in range(8):
        b = core // 2
        out[b] += results[core]["out"]
    return out


def kernel(x, w_attn, b_attn, w_proj, b_proj):
    x = np.asarray(x, np.float32)
    w_attn = np.asarray(w_attn, np.float32)
    b_attn = np.asarray(b_attn, np.float32)
    w_proj = np.asarray(w_proj, np.float32)
    b_proj = np.asarray(b_proj, np.float32)
    in_maps = shard_inputs(x, w_attn, b_attn, w_proj, b_proj)
    res = run_sharded(in_maps, trace=False)
    return gather(res.results)


# revision 8
# speedup vs baseline: 1.1685x; 1.1685x over previous
"""Causal self-attention (GPT-2 small block shape: B=4, T=2048, C=768, H=12, D=64)
on 8 TRN2 NeuronCores.

Sharding: core i handles batch b = i//2 and head-half = i%2 (6 heads each).
No cross-core collectives; the two half-head partial output projections per
batch are summed on the host during unshard (row-parallel c_proj).

Device kernel (per core, all matmuls bf16, fp32 PSUM accumulation):
  1. qkv^T = w^T x^T via TensorE with contraction over C (x^T supplied
     pre-transposed + bf16 by the host); biases folded in as K=1 matmuls
     with a ones vector.  Q is pre-scaled by 1/sqrt(D) host-side.
  2. Per head: S^T[k,q] = K^T.T @ Q^T blocks (causally skipped), exp on
     ScalarE with a large free dim straight out of PSUM into bf16 SBUF,
     diagonal-block masking on VectorE.
  3. AV with V augmented by a ones column -> row sums land in PSUM
     partition 64 for free; normalization = reciprocal (DVE) +
     K=1 broadcast matmul + one tensor-tensor multiply.
  4. Output projection from the y^T layout (contraction over the head dim),
     b_proj added via K=1 matmul (only on half 0; half 1 gets zeros).
"""

import sys

if "/opt/trn_rl_repo" not in sys.path:
    sys.path.insert(0, "/opt/trn_rl_repo")

import numpy as np
import ml_dtypes

import concourse.bass as bass  # noqa: F401  (engine types pulled via nc)
import concourse.mybir as mybir
from concourse import bacc
from concourse.tile import TileContext
from concourse.bass_utils import run_bass_kernel_spmd

BF16 = ml_dtypes.bfloat16

B, T, C = 4, 2048, 768
H, D = 12, 64
NH = 6  # heads per core
P = 128
TC = T // P  # 16 t-chunks of 128
QC = T // 512  # 4 q-chunks of 512
CCH = C // P  # 6 contraction chunks

DT = mybir.dt.bfloat16
F32 = mybir.dt.float32


def build_nc():
    nc = bacc.Bacc()

    xt_d = nc.declare_dram_parameter("xt", [P, CCH, T], DT, isOutput=False)
    wqk_d = nc.declare_dram_parameter("wqk", [P, CCH, 2 * NH * D], DT, isOutput=False)
    bqk_d = nc.declare_dram_parameter("bqk", [P, 2 * NH * D // P], F32, isOutput=False)
    wv_d = nc.declare_dram_parameter("wv", [P, CCH, NH * D], DT, isOutput=False)
    bv_d = nc.declare_dram_parameter("bv", [1, NH * D], DT, isOutput=False)
    wp_d = nc.declare_dram_parameter("wp", [P, NH * D // P, C], DT, isOutput=False)
    bp_d = nc.declare_dram_parameter("bp", [1, C], DT, isOutput=False)
    mask_d = nc.declare_dram_parameter("mask", [P, P], DT, isOutput=False)
    out_d = nc.declare_dram_parameter("out", [T, C], F32, isOutput=True)

    with TileContext(nc) as tc:
        with (
            tc.tile_pool(name="consts", bufs=1) as consts,
            tc.tile_pool(name="work", bufs=3) as work,
            tc.tile_pool(name="outp", bufs=3) as outp,
            tc.tile_pool(name="ps_s", bufs=2, space="PSUM") as ps_s,
            tc.tile_pool(name="ps_qkv", bufs=2, space="PSUM") as ps_qkv,
            tc.tile_pool(name="ps_av", bufs=2, space="PSUM") as ps_av,
        ):
            # ---- load inputs ----
            xt_sb = consts.tile([P, CCH, T], DT)
            nc.sync.dma_start(xt_sb[:, :, 0 : T // 4], xt_d[:, :, 0 : T // 4])
            wqk_sb = consts.tile([P, CCH, 2 * NH * D], DT)
            nc.sync.dma_start(wqk_sb[:], wqk_d[:])
            bqk_sb = consts.tile([P, 2 * NH * D // P], F32)
            nc.sync.dma_start(bqk_sb[:], bqk_d[:])
            for _q in range(1, 4):
                nc.sync.dma_start(
                    xt_sb[:, :, _q * T // 4 : (_q + 1) * T // 4],
                    xt_d[:, :, _q * T // 4 : (_q + 1) * T // 4],
                )
            wv_sb = consts.tile([P, CCH, NH * D], DT)
            nc.sync.dma_start(wv_sb[:], wv_d[:])
            bv_sb = consts.tile([1, NH * D], DT)
            nc.sync.dma_start(bv_sb[:], bv_d[:])
            wp_sb = consts.tile([P, NH * D // P, C], DT)
            nc.sync.dma_start(wp_sb[:], wp_d[:])
            bp_sb = consts.tile([1, C], DT)
            nc.sync.dma_start(bp_sb[:], bp_d[:])
            mask_sb = consts.tile([P, P], DT)
            nc.sync.dma_start(mask_sb[:], mask_d[:])

            bvb = consts.tile([P, NH * D], DT)
            nc.gpsimd.partition_broadcast(bvb[:], bv_sb[:])
            bpb = consts.tile([P, C], DT)
            nc.gpsimd.partition_broadcast(bpb[:], bp_sb[:])

            # Q^T/K^T as head-pair tiles [128, T]: head 2p in partitions 0:64,
            # head 2p+1 in partitions 64:128 (S-gen uses tile_position row=64)
            qtp = [consts.tile([P, T], DT, name=f"qtp{p}", tag=f"qtp{p}") for p in range(NH // 2)]
            ktp = [consts.tile([P, T], DT, name=f"ktp{p}", tag=f"ktp{p}") for p in range(NH // 2)]
            # V per t-chunk, heads side by side with a ones column: [128, 6, 65]
            vt = [consts.tile([P, NH, D + 1], DT, name=f"vt{t}", tag=f"vt{t}") for t in range(TC)]
            for t in range(TC):
                nc.gpsimd.memset(vt[t][:, :, D : D + 1], 1.0)
            # y^T per head-pair [128, T] bf16
            yt = [consts.tile([P, T], DT, name=f"yt{p}", tag=f"yt{p}") for p in range(NH // 2)]

            # ---- QKV helpers ----
            def qk_tile(fc, tcb):
                    # feat chunk fc: 0..2 -> Q pair fc, 3..5 -> K pair fc-3
                    pq = ps_qkv.tile([P, 512], F32, tag="qkv", name="pq")
                    for cc in range(CCH):
                        nc.tensor.matmul(
                            pq[:],
                            wqk_sb[:, cc, fc * P : (fc + 1) * P],
                            xt_sb[:, cc, tcb * 512 : (tcb + 1) * 512],
                            start=(cc == 0),
                            stop=(cc == CCH - 1),
                        )
                    dst = qtp[fc] if fc < 3 else ktp[fc - 3]
                    nc.vector.tensor_scalar_add(
                        dst[:, tcb * 512 : (tcb + 1) * 512],
                        pq[:],
                        bqk_sb[:, fc : fc + 1],
                    )

            def v_chunks(ts_):
                for t in ts_:
                    pv = ps_qkv.tile([P, NH * D], F32, tag="qkv", name="pv")
                    for cc in range(CCH):
                        nc.tensor.matmul(
                            pv[:],
                            xt_sb[:, cc, t * P : (t + 1) * P],
                            wv_sb[:, cc, :],
                            start=(cc == 0),
                            stop=(cc == CCH - 1),
                        )
                    nc.vector.tensor_add(
                        vt[t][:, :, 0:D],
                        pv[:].rearrange("p (h d) -> p h d", d=D),
                        bvb[:].rearrange("p (h d) -> p h d", d=D),
                    )

            # ---- attention for one head pair ----
            def attention(hp, qc):
                    nj = 4 * (qc + 1)  # causal k-chunks for this q block
                    for hi in (0, 1):
                        b0 = 64 * hi
                        av = ps_av.tile([65, 512], F32, tag="av", name="av")
                        for g in range(nj // 2):  # groups of 2 k-chunks
                            sps = ps_s.tile([P, 1024], F32, tag="s", name="sps")
                            for sub in range(2):
                                j = 2 * g + sub
                                m = max(0, (j - 4 * qc) * P)  # causal trim offset
                                nc.tensor.matmul(
                                    sps[:, sub * 512 + m : (sub + 1) * 512],
                                    ktp[hp][b0 : b0 + 64, j * P : (j + 1) * P],
                                    qtp[hp][b0 : b0 + 64, qc * 512 + m : (qc + 1) * 512],
                                    start=True,
                                    stop=True,
                                )
                            sexp = work.tile([P, 1024], DT, tag="sexp", name="sexp")
                            nc.scalar.activation(
                                sexp[:], sps[:], mybir.ActivationFunctionType.Exp
                            )
                            for sub in range(2):
                                j = 2 * g + sub
                                m = max(0, (j - 4 * qc) * P)
                                if j - 4 * qc >= 0:  # diagonal block: mask
                                    nc.vector.tensor_mul(
                                        sexp[:, sub * 512 + m : sub * 512 + m + P],
                                        sexp[:, sub * 512 + m : sub * 512 + m + P],
                                        mask_sb[:],
                                    )
                                nc.tensor.matmul(
                                    av[:, m:512],
                                    vt[j][:, 2 * hp + hi, :],
                                    sexp[:, sub * 512 + m : (sub + 1) * 512],
                                    start=(j == 0),
                                    stop=(j == nj - 1),
                                    skip_group_check=True,
                                )
                        # normalization: row sums sit in av partition 64
                        sums = work.tile([1, 512], F32, tag="sums", name="sums")
                        nc.vector.tensor_copy(sums[:], av[64:65, :])
                        inv = work.tile([1, 512], F32, tag="inv", name="inv")
                        nc.vector.reciprocal_approx_fast(inv[:], sums[:])
                        invb = work.tile([64, 512], F32, tag="invb", name="invb")
                        nc.gpsimd.partition_broadcast(invb[:], inv[:])
                        nc.vector.tensor_mul(
                            yt[hp][b0 : b0 + 64, qc * 512 : (qc + 1) * 512],
                            av[0:64, :],
                            invb[:],
                        )

            # ---- output projection for a batch of t chunks ----
            def proj(ts_):
                for t in ts_:
                    ppa = ps_qkv.tile([P, 512], F32, tag="qkv", name="ppa")
                    ppb = ps_av.tile([P, C - 512], F32, tag="av", name="ppb")
                    for pp, c0, c1 in ((ppa, 0, 512), (ppb, 512, C)):
                        for cp in range(NH * D // P):
                            nc.tensor.matmul(
                                pp[:, 0 : c1 - c0],
                                yt[cp][:, t * P : (t + 1) * P],
                                wp_sb[:, cp, c0:c1],
                                start=(cp == 0),
                                stop=(cp == NH * D // P - 1),
                            )
                    stg = outp.tile([P, C], F32)
                    nc.vector.tensor_add(stg[:, 0:512], ppa[:], bpb[:, 0:512])
                    nc.vector.tensor_add(stg[:, 512:C], ppb[:], bpb[:, 512:C])
                    nc.sync.dma_start(out_d[t * P : (t + 1) * P, :], stg[:])

            # ---- schedule: 4 uniform rounds over q blocks; proj lags a round
            for qc in range(QC):
                for fc in (0, 3, 1, 4, 2, 5):
                    qk_tile(fc, qc)
                v_chunks(range(4 * qc, 4 * qc + 4))
                if qc > 0:
                    proj(range(4 * (qc - 1), 4 * qc))
                for hp in range(3):
                    attention(hp, qc)
            proj(range(4 * (QC - 1), 4 * QC))

    nc.finalize()
    return nc

def shard_inputs(x, w_attn, b_attn, w_proj, b_proj):
    """Host-side prep: slice per core, transpose x, cast to bf16."""
    scale = 1.0 / np.sqrt(D)
    tril = np.tril(np.ones((P, P), np.float32))
    # mask[k_local, q_local] = 1 where k <= q
    mask = tril.T.astype(BF16)
    in_maps = []
    for core in range(8):
        b, half = divmod(core, 2)
        h0 = half * NH
        cq = slice(h0 * D, (h0 + NH) * D)
        ck = slice(C + h0 * D, C + (h0 + NH) * D)
        cv = slice(2 * C + h0 * D, 2 * C + (h0 + NH) * D)
        wq = (w_attn[:, cq] * scale).astype(BF16)
        wk = w_attn[:, ck].astype(BF16)
        wqk = np.concatenate([wq, wk], axis=1)  # [C, 768]
        bqk = np.concatenate([(b_attn[cq] * scale), b_attn[ck]], axis=0).astype(
            np.float32
        )
        bqk_col = np.ascontiguousarray(bqk.reshape(2 * NH * D // P, P).T)
        wv = w_attn[:, cv].astype(BF16)
        bv = b_attn[cv].astype(BF16)[None, :]
        wp = w_proj[h0 * D : (h0 + NH) * D, :].astype(BF16)
        bp = (b_proj if half == 0 else np.zeros_like(b_proj)).astype(BF16)[None, :]
        xt = np.ascontiguousarray(x[b].T).astype(BF16)  # [C, T]
        in_maps.append(
            {
                "xt": np.ascontiguousarray(
                    xt.reshape(CCH, P, T).transpose(1, 0, 2)
                ),
                "wqk": np.ascontiguousarray(
                    wqk.reshape(CCH, P, 2 * NH * D).transpose(1, 0, 2)
                ),
                "bqk": bqk_col,
                "wv": np.ascontiguousarray(
                    wv.reshape(CCH, P, NH * D).transpose(1, 0, 2)
                ),
                "bv": bv,
                "wp": np.ascontiguousarray(
                    wp.reshape(NH * D // P, P, C).transpose(1, 0, 2)
                ),
                "bp": bp,
                "mask": mask,
            }
        )
    return in_maps


_NC = None


def _get_nc():
    global _NC
    if _NC is None:
        _NC = build_nc()
    return _NC


def run_sharded(in_maps, trace=False, **kw):
    nc = _get_nc()
    return run_bass_kernel_spmd(nc, in_maps, core_ids=list(range(8)), trace=trace, **kw)


def gather(results):
    out = np.zeros((B, T, C), np.float32)
    for core in range(8):
        b = core // 2
        out[b] += results[core]["out"]
    return out


def kernel(x, w_attn, b_attn, w_proj, b_proj):
    x = np.asarray(x, np.float32)
    w_attn = np.asarray(w_attn, np.float32)
    b_attn = np.asarray(b_attn, np.float32)
    w_proj = np.asarray(w_proj, np.float32)
    b_proj = np.asarray(b_proj, np.float32)
    in_maps = shard_inputs(x, w_attn, b_attn, w_proj, b_proj)
    res = run_sharded(in_maps, trace=False)
    return gather(res.results)


# revision 9
# speedup vs baseline: 1.1848x; 1.0139x over previous
"""Causal self-attention (GPT-2 small block shape: B=4, T=2048, C=768, H=12, D=64)
on 8 TRN2 NeuronCores.

Sharding: core i handles batch b = i//2 and head-half = i%2 (6 heads each).
No cross-core collectives; the two half-head partial output projections per
batch are summed on the host during unshard (row-parallel c_proj).

Device kernel (per core, all matmuls bf16, fp32 PSUM accumulation):
  1. qkv^T = w^T x^T via TensorE with contraction over C (x^T supplied
     pre-transposed + bf16 by the host); biases folded in as K=1 matmuls
     with a ones vector.  Q is pre-scaled by 1/sqrt(D) host-side.
  2. Per head: S^T[k,q] = K^T.T @ Q^T blocks (causally skipped), exp on
     ScalarE with a large free dim straight out of PSUM into bf16 SBUF,
     diagonal-block masking on VectorE.
  3. AV with V augmented by a ones column -> row sums land in PSUM
     partition 64 for free; normalization = reciprocal (DVE) +
     K=1 broadcast matmul + one tensor-tensor multiply.
  4. Output projection from the y^T layout (contraction over the head dim),
     b_proj added via K=1 matmul (only on half 0; half 1 gets zeros).
"""

import sys

if "/opt/trn_rl_repo" not in sys.path:
    sys.path.insert(0, "/opt/trn_rl_repo")

import numpy as np
import ml_dtypes

import concourse.bass as bass  # noqa: F401  (engine types pulled via nc)
import concourse.mybir as mybir
from concourse import bacc
from concourse.tile import TileContext
from concourse.bass_utils import run_bass_kernel_spmd

BF16 = ml_dtypes.bfloat16

B, T, C = 4, 2048, 768
H, D = 12, 64
NH = 6  # heads per core
P = 128
TC = T // P  # 16 t-chunks of 128
QC = T // 512  # 4 q-chunks of 512
CCH = C // P  # 6 contraction chunks

DT = mybir.dt.bfloat16
F32 = mybir.dt.float32


def build_nc():
    nc = bacc.Bacc()

    xt_d = nc.declare_dram_parameter("xt", [P, CCH, T], DT, isOutput=False)
    wqk_d = nc.declare_dram_parameter("wqk", [P, CCH, 2 * NH * D], DT, isOutput=False)
    bqk_d = nc.declare_dram_parameter("bqk", [P, 2 * NH * D // P], F32, isOutput=False)
    wv_d = nc.declare_dram_parameter("wv", [P, CCH, NH * D], DT, isOutput=False)
    bv_d = nc.declare_dram_parameter("bv", [1, NH * D], DT, isOutput=False)
    wp_d = nc.declare_dram_parameter("wp", [P, NH * D // P, C], DT, isOutput=False)
    bp_d = nc.declare_dram_parameter("bp", [1, C], DT, isOutput=False)
    mask_d = nc.declare_dram_parameter("mask", [P, P], DT, isOutput=False)
    out_d = nc.declare_dram_parameter("out", [T, C], F32, isOutput=True)

    with TileContext(nc) as tc:
        with (
            tc.tile_pool(name="consts", bufs=1) as consts,
            tc.tile_pool(name="work", bufs=3) as work,
            tc.tile_pool(name="outp", bufs=3) as outp,
            tc.tile_pool(name="ps_s", bufs=2, space="PSUM") as ps_s,
            tc.tile_pool(name="ps_qkv", bufs=2, space="PSUM") as ps_qkv,
            tc.tile_pool(name="ps_av", bufs=2, space="PSUM") as ps_av,
        ):
            # ---- load inputs ----
            # inputs split across the two HWDGE queues (sync + scalar)
            xt_sb = consts.tile([P, CCH, T], DT)
            nc.sync.dma_start(xt_sb[:, :, 0 : T // 4], xt_d[:, :, 0 : T // 4])
            wqk_sb = consts.tile([P, CCH, 2 * NH * D], DT)
            nc.scalar.dma_start(wqk_sb[:], wqk_d[:])
            bqk_sb = consts.tile([P, 2 * NH * D // P], F32)
            nc.scalar.dma_start(bqk_sb[:], bqk_d[:])
            wv_sb = consts.tile([P, CCH, NH * D], DT)
            nc.sync.dma_start(wv_sb[:], wv_d[:])
            for _q in range(1, 4):
                eng = nc.scalar if _q % 2 else nc.sync
                eng.dma_start(
                    xt_sb[:, :, _q * T // 4 : (_q + 1) * T // 4],
                    xt_d[:, :, _q * T // 4 : (_q + 1) * T // 4],
                )
            bv_sb = consts.tile([1, NH * D], DT)
            nc.sync.dma_start(bv_sb[:], bv_d[:])
            wp_sb = consts.tile([P, NH * D // P, C], DT)
            nc.scalar.dma_start(wp_sb[:], wp_d[:])
            bp_sb = consts.tile([1, C], DT)
            nc.sync.dma_start(bp_sb[:], bp_d[:])
            mask_sb = consts.tile([P, P], DT)
            nc.sync.dma_start(mask_sb[:], mask_d[:])

            bvb = consts.tile([P, NH * D], DT)
            nc.gpsimd.partition_broadcast(bvb[:], bv_sb[:])
            bpb = consts.tile([P, C], DT)
            nc.gpsimd.partition_broadcast(bpb[:], bp_sb[:])

            # Q^T/K^T as head-pair tiles [128, T]: head 2p in partitions 0:64,
            # head 2p+1 in partitions 64:128 (S-gen uses tile_position row=64)
            qtp = [consts.tile([P, T], DT, name=f"qtp{p}", tag=f"qtp{p}") for p in range(NH // 2)]
            ktp = [consts.tile([P, T], DT, name=f"ktp{p}", tag=f"ktp{p}") for p in range(NH // 2)]
            # V per t-chunk, heads side by side with a ones column: [128, 6, 65]
            vt = [consts.tile([P, NH, D + 1], DT, name=f"vt{t}", tag=f"vt{t}") for t in range(TC)]
            for t in range(TC):
                nc.gpsimd.memset(vt[t][:, :, D : D + 1], 1.0)
            # y^T per head-pair [128, T] bf16
            yt = [consts.tile([P, T], DT, name=f"yt{p}", tag=f"yt{p}") for p in range(NH // 2)]

            # ---- QKV helpers ----
            def qk_tile(fc, tcb):
                    # feat chunk fc: 0..2 -> Q pair fc, 3..5 -> K pair fc-3
                    pq = ps_qkv.tile([P, 512], F32, tag="qkv", name="pq")
                    for cc in range(CCH):
                        nc.tensor.matmul(
                            pq[:],
                            wqk_sb[:, cc, fc * P : (fc + 1) * P],
                            xt_sb[:, cc, tcb * 512 : (tcb + 1) * 512],
                            start=(cc == 0),
                            stop=(cc == CCH - 1),
                        )
                    dst = qtp[fc] if fc < 3 else ktp[fc - 3]
                    nc.vector.tensor_scalar_add(
                        dst[:, tcb * 512 : (tcb + 1) * 512],
                        pq[:],
                        bqk_sb[:, fc : fc + 1],
                    )

            def v_chunks(ts_):
                for t in ts_:
                    pv = ps_qkv.tile([P, NH * D], F32, tag="qkv", name="pv")
                    for cc in range(CCH):
                        nc.tensor.matmul(
                            pv[:],
                            xt_sb[:, cc, t * P : (t + 1) * P],
                            wv_sb[:, cc, :],
                            start=(cc == 0),
                            stop=(cc == CCH - 1),
                        )
                    nc.vector.tensor_add(
                        vt[t][:, :, 0:D],
                        pv[:].rearrange("p (h d) -> p h d", d=D),
                        bvb[:].rearrange("p (h d) -> p h d", d=D),
                    )

            # ---- attention for one head pair ----
            def attention(hp, qc):
                    nj = 4 * (qc + 1)  # causal k-chunks for this q block
                    for hi in (0, 1):
                        b0 = 64 * hi
                        av = ps_av.tile([65, 512], F32, tag="av", name="av")
                        for g in range(nj // 2):  # groups of 2 k-chunks
                            sps = ps_s.tile([P, 1024], F32, tag="s", name="sps")
                            for sub in range(2):
                                j = 2 * g + sub
                                m = max(0, (j - 4 * qc) * P)  # causal trim offset
                                nc.tensor.matmul(
                                    sps[:, sub * 512 + m : (sub + 1) * 512],
                                    ktp[hp][b0 : b0 + 64, j * P : (j + 1) * P],
                                    qtp[hp][b0 : b0 + 64, qc * 512 + m : (qc + 1) * 512],
                                    start=True,
                                    stop=True,
                                )
                            sexp = work.tile([P, 1024], DT, tag="sexp", name="sexp")
                            nc.scalar.activation(
                                sexp[:], sps[:], mybir.ActivationFunctionType.Exp
                            )
                            for sub in range(2):
                                j = 2 * g + sub
                                m = max(0, (j - 4 * qc) * P)
                                if j - 4 * qc >= 0:  # diagonal block: mask
                                    nc.vector.tensor_mul(
                                        sexp[:, sub * 512 + m : sub * 512 + m + P],
                                        sexp[:, sub * 512 + m : sub * 512 + m + P],
                                        mask_sb[:],
                                    )
                                nc.tensor.matmul(
                                    av[:, m:512],
                                    vt[j][:, 2 * hp + hi, :],
                                    sexp[:, sub * 512 + m : (sub + 1) * 512],
                                    start=(j == 0),
                                    stop=(j == nj - 1),
                                    skip_group_check=True,
                                )
                        # normalization: row sums sit in av partition 64
                        sums = work.tile([1, 512], F32, tag="sums", name="sums")
                        nc.vector.tensor_copy(sums[:], av[64:65, :])
                        inv = work.tile([1, 512], F32, tag="inv", name="inv")
                        nc.vector.reciprocal_approx_fast(inv[:], sums[:])
                        invb = work.tile([64, 512], F32, tag="invb", name="invb")
                        nc.gpsimd.partition_broadcast(invb[:], inv[:])
                        nc.vector.tensor_mul(
                            yt[hp][b0 : b0 + 64, qc * 512 : (qc + 1) * 512],
                            av[0:64, :],
                            invb[:],
                        )

            # ---- output projection for a batch of t chunks ----
            def proj(ts_):
                for t in ts_:
                    ppa = ps_qkv.tile([P, 512], F32, tag="qkv", name="ppa")
                    ppb = ps_av.tile([P, C - 512], F32, tag="av", name="ppb")
                    for pp, c0, c1 in ((ppa, 0, 512), (ppb, 512, C)):
                        for cp in range(NH * D // P):
                            nc.tensor.matmul(
                                pp[:, 0 : c1 - c0],
                                yt[cp][:, t * P : (t + 1) * P],
                                wp_sb[:, cp, c0:c1],
                                start=(cp == 0),
                                stop=(cp == NH * D // P - 1),
                            )
                    stg = outp.tile([P, C], F32)
                    nc.vector.tensor_add(stg[:, 0:512], ppa[:], bpb[:, 0:512])
                    nc.vector.tensor_add(stg[:, 512:C], ppb[:], bpb[:, 512:C])
                    eng = nc.scalar if t % 2 else nc.sync
                    eng.dma_start(out_d[t * P : (t + 1) * P, :], stg[:])

            # ---- schedule: 4 uniform rounds over q blocks; proj lags a round
            for qc in range(QC):
                for fc in (0, 3, 1, 4, 2, 5):
                    qk_tile(fc, qc)
                v_chunks(range(4 * qc, 4 * qc + 4))
                if qc > 0:
                    proj(range(4 * (qc - 1), 4 * qc))
                for hp in range(3):
                    attention(hp, qc)
            proj(range(4 * (QC - 1), 4 * QC))

    nc.finalize()
    return nc

def shard_inputs(x, w_attn, b_attn, w_proj, b_proj):
    """Host-side prep: slice per core, transpose x, cast to bf16."""
    scale = 1.0 / np.sqrt(D)
    tril = np.tril(np.ones((P, P), np.float32))
    # mask[k_local, q_local] = 1 where k <= q
    mask = tril.T.astype(BF16)
    in_maps = []
    for core in range(8):
        b, half = divmod(core, 2)
        h0 = half * NH
        cq = slice(h0 * D, (h0 + NH) * D)
        ck = slice(C + h0 * D, C + (h0 + NH) * D)
        cv = slice(2 * C + h0 * D, 2 * C + (h0 + NH) * D)
        wq = (w_attn[:, cq] * scale).astype(BF16)
        wk = w_attn[:, ck].astype(BF16)
        wqk = np.concatenate([wq, wk], axis=1)  # [C, 768]
        bqk = np.concatenate([(b_attn[cq] * scale), b_attn[ck]], axis=0).astype(
            np.float32
        )
        bqk_col = np.ascontiguousarray(bqk.reshape(2 * NH * D // P, P).T)
        wv = w_attn[:, cv].astype(BF16)
        bv = b_attn[cv].astype(BF16)[None, :]
        wp = w_proj[h0 * D : (h0 + NH) * D, :].astype(BF16)
        bp = (b_proj if half == 0 else np.zeros_like(b_proj)).astype(BF16)[None, :]
        xt = np.ascontiguousarray(x[b].T).astype(BF16)  # [C, T]
        in_maps.append(
            {
                "xt": np.ascontiguousarray(
                    xt.reshape(CCH, P, T).transpose(1, 0, 2)
                ),
                "wqk": np.ascontiguousarray(
                    wqk.reshape(CCH, P, 2 * NH * D).transpose(1, 0, 2)
                ),
                "bqk": bqk_col,
                "wv": np.ascontiguousarray(
                    wv.reshape(CCH, P, NH * D).transpose(1, 0, 2)
                ),
                "bv": bv,
                "wp": np.ascontiguousarray(
                    wp.reshape(NH * D // P, P, C).transpose(1, 0, 2)
                ),
                "bp": bp,
                "mask": mask,
            }
        )
    return in_maps


_NC = None


def _get_nc():
    global _NC
    if _NC is None:
        _NC = build_nc()
    return _NC


def run_sharded(in_maps, trace=False, **kw):
    nc = _get_nc()
    return run_bass_kernel_spmd(nc, in_maps, core_ids=list(range(8)), trace=trace, **kw)


def gather(results):
    out = np.zeros((B, T, C), np.float32)
    for core in range(8):
        b = core // 2
        out[b] += results[core]["out"]
    return out


def kernel(x, w_attn, b_attn, w_proj, b_proj):
    x = np.asarray(x, np.float32)
    w_attn = np.asarray(w_attn, np.float32)
    b_attn = np.asarray(b_attn, np.float32)
    w_proj = np.asarray(w_proj, np.float32)
    b_proj = np.asarray(b_proj, np.float32)
    in_maps = shard_inputs(x, w_attn, b_attn, w_proj, b_proj)
    res = run_sharded(in_maps, trace=False)
    return gather(res.results)
